# revision 3
# baseline (speedup 1.0000x reference)
"""Fused single-head attention (QKV projection + softmax(QK^T)V) on 8 trn2 cores.

Problem (hardcoded): x [4, 4096, 768] f32, W_qkv [768, 2304] f32, b_qkv [2304] f32.
  qkv = x @ W_qkv + b_qkv ; q,k,v = split(qkv, 3)
  out = softmax(q k^T / sqrt(768)) v          -> [4, 4096, 768] f32

Sharding: batch (4) x key-halves (2) -> 8 cores, no cross-core traffic.
Each core gets one batch's x (pre-transposed on host to xT [768, 4096] fp16,
with the key half it owns rotated into columns [0, 2048)), projects q for
all 4096 queries but k/v only for its 2048 keys, and computes PARTIAL
attention sums over those keys. The host combines the two partials of each
pair: (n0 + n1) / (d0 + d1). No max-subtraction: scores are O(1) here.

fp8 strategy (the speed lever -- PE fp8 DoubleRow = 2x fp16 rate):
  - Projection stays fp16 (fp8 proj error 2.8e-2 > 2e-2 gate).
  - QK^T in fp8 DoubleRow: q,k quantized e4m3 by the projection's bias
    activation. Adds ~1.3e-2 output error (simulated; gate is 2e-2).
  - PV in fp8 DoubleRow with two error tricks (simulated total 1.43e-2):
      * p' = fp8(exp(s) - 1) instead of fp8(exp(s)): p~1 so centering
        shrinks quantization error ~3x. Numerator correction
        colsum_v = sum_j v8_j is computed once per core by a ones-stationary
        matmul and added back on the host.
      * v split into v_hi + v_lo (fp8 pair, lo = residual): v error -> ~0.
        Both PV matmuls share the p' stationary.
  - Softmax denominator: an extra DoubleRow matmul per (i-tile, j-pair)
    with the same p' stationary and a ones moving vector accumulates
    den' = sum_j p' into output column 768; host adds back 2048.

On-chip layout:
  - qT8 [h(128), ht(6), i(4096)] fp8, kT8 [h, ht, j(2048)] fp8 -- h-major so
    QK contracts over partitions; DoubleRow pairs adjacent ht tiles.
  - scores computed transposed: s[j(128), i] = kT-tile.T @ qT -> PSUM
    [128, 1024] (an i-block pair), exp via ScalarE -> p16, p' = p16 - 1 on
    VectorE -> pp8 [j(128), jt(16), i(1024)] fp8.
  - PV: stationary p' [j, 2, i-tile(128)], moving v8 [j, 2, h] -> og
    [i(128), 1024] PSUM: cols 0:768 numerator, col 768 denominator.
  - i-block pairs (1024 queries) let the QK stationary k-tile be reused
    across 2 i-blocks (MM-bound instead of LDW-bound).
PSUM (8 banks): scores 2x[128,1024] (4) + og 2x[128,1024] (4).
"""

import math
from contextlib import ExitStack
from functools import lru_cache

import numpy as np

import concourse.bacc as bacc
import concourse.bass as bass
import concourse.tile as tile
from concourse import mybir
from concourse.bass_utils import run_bass_kernel_spmd

B, N, C = 4, 4096, 768
H = 768          # head dim (== C)
H3 = 3 * H
NCORES = 8
NK = N // 2      # keys per core
DT = mybir.dt.float16
F8 = mybir.dt.float8e4
F32 = mybir.dt.float32
SCALE = 1.0 / math.sqrt(H)
DR = mybir.MatmulPerfMode.DoubleRow

CT = C // 128    # 6 contraction tiles (c) for the projection
HT = H // 128    # 6 head tiles (h)
HP = HT // 2     # 3 DoubleRow head-tile pairs
JT = NK // 128   # 16 key tiles (j) per core
JP = JT // 2     # 8 DoubleRow key-tile pairs
RB = 8           # r-blocks of 512 over the 4096 rows (projection)
RBS = N // RB    # 512
KRB = RB // 2    # r-blocks that contain this core's keys (first 4)
IBP = 4          # i-block pairs of 1024 queries (attention)
IPS = N // IBP   # 1024
ITS = IPS // 128  # 8 i-tiles of 128 per i-block pair


def build_program():
    nc = bacc.Bacc(
        "TRN2",
        target_bir_lowering=False,
        debug=False,
        enable_asserts=False,
        num_devices=NCORES,
    )
    xT_d = nc.dram_tensor("xT", [C, N], DT, kind="ExternalInput").ap()
    w_d = nc.dram_tensor("w", [C, H3], DT, kind="ExternalInput").ap()
    bqk_d = nc.dram_tensor("bqk", [128, 2 * HT], F32, kind="ExternalInput").ap()
    bv_d = nc.dram_tensor("bv", [128, H], F32, kind="ExternalInput").ap()
    # out[:, 0:768] = sum_j p'_j v8_j  (numerator, minus colsum correction)
    # out[:, 768]   = sum_j p'_j      (denominator - 2048)
    out_d = nc.dram_tensor("out", [N, H + 1], DT, kind="ExternalOutput").ap()
    colsum_d = nc.dram_tensor("colsum", [1, H], F32, kind="ExternalOutput").ap()

    with tile.TileContext(nc) as tc:
        with ExitStack() as ctx:
            persist = ctx.enter_context(tc.tile_pool(name="persist", bufs=1))

            qT8 = persist.tile([128, HT, N], F8, tag="qT8")
            kT8 = persist.tile([128, HT, NK], F8, tag="kT8")
            vhi = persist.tile([128, JT, H], F8, tag="vhi")
            vlo = persist.tile([128, JT, H], F8, tag="vlo")
            pp8 = persist.tile([128, JT, IPS], F8, tag="pp8")
            ones8 = persist.tile([128, 2, 16], F8, tag="ones8")
            bqk = persist.tile([128, 2 * HT], F32, tag="bqk")
            bvb = persist.tile([128, H], F32, tag="bvb")

            # ---- Phase 1: QKV projection (fp16 in, fp8 out) ----
            with tc.tile_pool(name="wpool", bufs=1) as wpool, \
                 tc.tile_pool(name="xpool", bufs=3 * CT) as xpool, \
                 tc.tile_pool(name="vtmp", bufs=2) as vtmp, \
                 tc.tile_pool(name="pj", bufs=4, space="PSUM") as pj, \
                 tc.tile_pool(name="pv", bufs=2, space="PSUM") as pv:

                ws = [wpool.tile([128, H3], DT, tag=f"w{t}", name=f"w{t}")
                      for t in range(CT)]

                def load_xt(rb):
                    r0 = rb * RBS
                    tiles = []
                    for ct in range(CT):
                        t = xpool.tile([128, RBS], DT, tag="xt", name=f"xt{rb}_{ct}")
                        nc.sync.dma_start(
                            out=t, in_=xT_d[ct * 128:(ct + 1) * 128, r0:r0 + RBS])
                        tiles.append(t)
                    return tiles

                # DMA issue order = need order: k-projection h-tile-0 columns
                # of W interleaved with the first r-block's x, then biases,
                # then the rest of W.
                xts = [None] * RB
                xt0 = []
                for ct in range(CT):
                    nc.sync.dma_start(
                        out=ws[ct][:, H: H + 128],
                        in_=w_d[ct * 128:(ct + 1) * 128, H: H + 128])
                    t = xpool.tile([128, RBS], DT, tag="xt", name=f"xt0_{ct}")
                    nc.sync.dma_start(out=t, in_=xT_d[ct * 128:(ct + 1) * 128, 0:RBS])
                    xt0.append(t)
                xts[0] = xt0
                for ht in range(1, HT):
                    for ct in range(CT):
                        nc.sync.dma_start(
                            out=ws[ct][:, H + ht * 128: H + (ht + 1) * 128],
                            in_=w_d[ct * 128:(ct + 1) * 128,
                                    H + ht * 128: H + (ht + 1) * 128])
                    if ht == 1:
                        nc.sync.dma_start(out=bqk, in_=bqk_d)
                for ct in range(CT):
                    nc.sync.dma_start(out=ws[ct][:, 0:H],
                                      in_=w_d[ct * 128:(ct + 1) * 128, 0:H])
                    nc.sync.dma_start(out=ws[ct][:, 2 * H:H3],
                                      in_=w_d[ct * 128:(ct + 1) * 128, 2 * H:H3])
                nc.sync.dma_start(out=bvb, in_=bv_d)

                # PE warm-up: ~3.4us of junk matmuls (no DMA deps) so the
                # HAM clock-gate reaches full rate while the first x/W DMAs
                # are still in flight.
                warm_l = xpool.tile([128, 128], DT, tag="warml", name="warml")
                warm_r = xpool.tile([128, 512], DT, tag="warmr", name="warmr")
                nc.vector.memset(warm_l, 0.0)
                nc.vector.memset(warm_r, 0.0)
                nc.vector.memset(ones8, 1.0)
                for i in range(16):
                    wp = pj.tile([128, RBS], F32, tag="pj", name=f"warm{i}")
                    nc.tensor.matmul(wp, warm_l, warm_r, start=True, stop=True)

                for rb in range(RB):
                    r0 = rb * RBS
                    if rb + 1 < RB:
                        xts[rb + 1] = load_xt(rb + 1)
                    xt = xts[rb]

                    projs = [(0, qT8, r0)]             # q: every r-block
                    if rb < KRB:
                        projs.insert(0, (H, kT8, r0))  # k: first half only
                    for (wofs, dst, c0) in projs:
                        for ht in range(HT):
                            ps = pj.tile([128, RBS], F32, tag="pj")
                            for ct in range(CT):
                                nc.tensor.matmul(
                                    ps,
                                    ws[ct][:, wofs + ht * 128: wofs + (ht + 1) * 128],
                                    xt[ct],
                                    start=(ct == 0), stop=(ct == CT - 1),
                                )
                            bcol = (0 if wofs == 0 else HT) + ht
                            nc.scalar.activation(
                                out=dst[:, ht, c0:c0 + RBS],
                                in_=ps,
                                func=mybir.ActivationFunctionType.Identity,
                                bias=bqk[:, bcol:bcol + 1],
                            )

                    if rb < KRB:
                        for j in range(RBS // 128):
                            jt = rb * (RBS // 128) + j
                            ps = pv.tile([128, H], F32, tag="pv")
                            for ct in range(CT):
                                xs = xt[ct][:, j * 128:(j + 1) * 128]
                                nc.tensor.matmul(
                                    ps[:, 0:512], xs, ws[ct][:, 2 * H: 2 * H + 512],
                                    start=(ct == 0), stop=(ct == CT - 1))
                                nc.tensor.matmul(
                                    ps[:, 512:H], xs, ws[ct][:, 2 * H + 512: 3 * H],
                                    start=(ct == 0), stop=(ct == CT - 1))
                            s1 = vtmp.tile([128, H], DT, tag="s1")
                            nc.vector.tensor_add(s1, ps, bvb)
                            nc.vector.tensor_copy(out=vhi[:, jt, :], in_=s1)
                            nc.vector.tensor_sub(vlo[:, jt, :], s1, vhi[:, jt, :])

                # colsum_v8[h] = sum_j (vhi + vlo)[j, h] via ones-stationary
                # DoubleRow matmuls; reuses the pv psum banks.
                csum = vtmp.tile([1, H], F32, tag="csum")
                for seg0, seg1 in ((0, 512), (512, H)):
                    cs_ps = pv.tile([1, 512], F32, tag="pv", name=f"cs{seg0}")
                    for jp in range(JP):
                        for vi, vt in enumerate((vhi, vlo)):
                            nc.tensor.matmul(
                                cs_ps[:, 0:seg1 - seg0],
                                ones8[:, :, 0:1],
                                vt[:, 2 * jp:2 * jp + 2, seg0:seg1],
                                start=(jp == 0 and vi == 0),
                                stop=(jp == JP - 1 and vi == 1),
                                perf_mode=DR,
                            )
                    nc.vector.tensor_copy(out=csum[:, seg0:seg1],
                                          in_=cs_ps[:, 0:seg1 - seg0])
                nc.sync.dma_start(out=colsum_d, in_=csum)

            # ---- Phase 2: attention (partial sums over this core's keys) ----
            with tc.tile_pool(name="p16pool", bufs=2) as p16pool, \
                 tc.tile_pool(name="opool", bufs=4) as opool, \
                 tc.tile_pool(name="ps_s", bufs=2, space="PSUM") as ps_s, \
                 tc.tile_pool(name="ps_o", bufs=2, space="PSUM") as ps_o:

                for ibp in range(IBP):
                    i0 = ibp * IPS
                    # Stage A: scores + exp + p' for this i-block pair
                    for jt in range(JT):
                        sps = ps_s.tile([128, IPS], F32, tag="s")
                        for hp in range(HP):
                            kslice = kT8[:, 2 * hp:2 * hp + 2,
                                         jt * 128:(jt + 1) * 128]
                            for half in range(2):
                                nc.tensor.matmul(
                                    sps[:, half * 512:(half + 1) * 512],
                                    kslice,
                                    qT8[:, 2 * hp:2 * hp + 2,
                                        i0 + half * 512: i0 + (half + 1) * 512],
                                    start=(hp == 0), stop=(hp == HP - 1),
                                    perf_mode=DR,
                                )
                        p16 = p16pool.tile([128, IPS], DT, tag="p16")
                        nc.scalar.activation(
                            out=p16, in_=sps,
                            func=mybir.ActivationFunctionType.Exp,
                            scale=SCALE,
                        )
                        nc.vector.tensor_scalar_sub(pp8[:, jt, :], p16, 1.0)

                    # Stage B: PV per i-tile; den' into column 768
                    for it in range(ITS):
                        og = ps_o.tile([128, IPS], F32, tag="og",
                                       name=f"og{ibp}_{it}")
                        for jp in range(JP):
                            pslice = pp8[:, 2 * jp:2 * jp + 2,
                                         it * 128:(it + 1) * 128]
                            st = (jp == 0)
                            sp = (jp == JP - 1)
                            nc.tensor.matmul(
                                og[:, 0:512], pslice,
                                vhi[:, 2 * jp:2 * jp + 2, 0:512],
                                start=st, stop=False, perf_mode=DR)
                            nc.tensor.matmul(
                                og[:, 512:H], pslice,
                                vhi[:, 2 * jp:2 * jp + 2, 512:H],
                                start=st, stop=False, perf_mode=DR)
                            # NO start=True here: PSUM zeroing is bank-granular
                            # (2KB); the hi-seg2 matmul's start already marks
                            # the whole bank pending-zero, which zeroes these
                            # bytes on first touch. A start here would re-mark
                            # the bank and wipe hi-seg2's jp0 contribution.
                            nc.tensor.matmul(
                                og[:, H:H + 1], pslice,
                                ones8[:, :, 0:1],
                                start=False, stop=sp, perf_mode=DR,
                                skip_group_check=True)
                            nc.tensor.matmul(
                                og[:, 0:512], pslice,
                                vlo[:, 2 * jp:2 * jp + 2, 0:512],
                                start=False, stop=sp, perf_mode=DR)
                            nc.tensor.matmul(
                                og[:, 512:H], pslice,
                                vlo[:, 2 * jp:2 * jp + 2, 512:H],
                                start=False, stop=sp, perf_mode=DR)
                        ot = opool.tile([128, H + 1], DT, tag="ot",
                                        name=f"ot{ibp}_{it}")
                        nc.scalar.activation(
                            out=ot[:, 0:384], in_=og[:, 0:384],
                            func=mybir.ActivationFunctionType.Copy)
                        nc.vector.tensor_copy(out=ot[:, 384:H + 1],
                                              in_=og[:, 384:H + 1])
                        r0 = i0 + it * 128
                        dma = nc.scalar.dma_start if it % 2 else nc.sync.dma_start
                        dma(out=out_d[r0:r0 + 128, :], in_=ot)
    nc.compile()
    return nc


@lru_cache(maxsize=1)
def _cached_program():
    return build_program()


def _prep_in_maps(x, W_qkv, b_qkv):
    x = np.asarray(x, dtype=np.float32)
    W_qkv = np.asarray(W_qkv, dtype=np.float32)
    b_qkv = np.asarray(b_qkv, dtype=np.float32)
    w16 = W_qkv.astype(np.float16)
    bq = b_qkv[0:H].astype(np.float32).reshape(HT, 128).T    # [128, HT]
    bk = b_qkv[H:2 * H].astype(np.float32).reshape(HT, 128).T
    bqk = np.ascontiguousarray(np.concatenate([bq, bk], axis=1))  # [128, 2*HT]
    bv = np.ascontiguousarray(
        np.broadcast_to(b_qkv[2 * H:3 * H].astype(np.float32), (128, H)))

    in_maps = []
    for core in range(NCORES):
        b, kh = core // 2, core % 2
        xb = x[b]  # [N, C] f32
        if kh == 1:
            # Rotate so this core's key rows occupy rows [0, NK). Queries are
            # also rotated; the host rotates this core's outputs back.
            xb = np.concatenate([xb[NK:], xb[:NK]], axis=0)
        xT = np.ascontiguousarray(xb.T).astype(np.float16)
        in_maps.append({"xT": xT, "w": w16, "bqk": bqk, "bv": bv})
    return in_maps


def _combine(results):
    out = np.empty((B, N, C), dtype=np.float32)
    for b in range(B):
        acc_n = None
        acc_d = None
        for kh in (0, 1):
            r = results[2 * b + kh]
            o = np.asarray(r["out"], dtype=np.float32)      # [N, H+1]
            cs = np.asarray(r["colsum"], dtype=np.float32)  # [1, H]
            num = o[:, 0:H] + cs                            # add colsum_v8
            den = o[:, H] + float(NK)                       # add sum_j 1
            if kh == 1:
                # core worked in rotated query order; rotate back
                num = np.concatenate([num[NK:], num[:NK]], axis=0)
                den = np.concatenate([den[NK:], den[:NK]])
            acc_n = num if acc_n is None else acc_n + num
            acc_d = den if acc_d is None else acc_d + den
        out[b] = acc_n / acc_d[:, None]
    return out


def kernel(x, W_qkv, b_qkv):
    nc = _cached_program()
    in_maps = _prep_in_maps(x, W_qkv, b_qkv)
    res = run_bass_kernel_spmd(nc, in_maps, core_ids=list(range(NCORES)))
    return _combine(res.results)


# revision 5
# speedup vs baseline: 1.4604x; 1.4604x over previous
"""Fused single-head attention (QKV projection + softmax(QK^T)V) on 8 trn2 cores.

Problem (hardcoded): x [4, 4096, 768] f32, W_qkv [768, 2304] f32, b_qkv [2304] f32.
  qkv = x @ W_qkv + b_qkv ; q,k,v = split(qkv, 3)
  out = softmax(q k^T / sqrt(768)) v          -> [4, 4096, 768] f32

Sharding: batch (4) x key-halves (2) -> 8 cores. Each core receives ONLY its
key half of x (xT [768, 2048] fp16, rows kh*2048:(kh+1)*2048 of the batch),
projects q/k/v for those 2048 rows, then an AllGather over the core pair
rebuilds the full q (4096 queries, ORIGINAL row order, identical on both
cores). Each core computes PARTIAL attention sums over its 2048 keys; the
host combines pairs: (n0 + n1) / (d0 + d1). No max-subtraction: scores O(1).

fp8 strategy (PE DoubleRow = 256-deep contraction per pass, 2x fp16 FLOPs;
matmul cost = out-columns x 1 cycle regardless of dtype):
  - Projection fp16 (fp8 proj error 2.8e-2 > 2e-2 gate).
  - QK^T fp8 DoubleRow (q,k e4m3 via the projection bias-activation).
  - PV fp8 DoubleRow with p' = fp8(exp(s)-1) (p~1 so centering shrinks the
    fp8 error ~3x; numerator correction colsum_v8 = sum_j v8_j added on
    host). v8 = v_hi + v_lo fp8 pair, but p'@v_lo is DROPPED (product of two
    small uncorrelated factors, ~6e-4 of output); v_lo feeds only colsum so
    v carries no bias. Simulated end-to-end error 1.62e-2 (gate 2e-2).
  - Softmax denominator: one extra DoubleRow matmul per (i-tile, j-pair)
    reusing the p' stationary with a ones moving vector -> output column 768;
    host adds back 2048.

On-chip layout:
  - qT8 [h(128), ht(6), i(4096)], kT8 [h, ht, j(2048)] fp8, h-major;
    DoubleRow pairs adjacent ht tiles.
  - scores transposed: s[j(128), i] = kT-tile.T @ qT -> PSUM [128, 1024]
    (i-block pair so the k stationary is reused), exp on ScalarE -> p16,
    p' = p16 - 1 on VectorE -> pp8 [j(128), jt(16), i(1024)] fp8.
  - PV: stationary p' [j, 2, i-tile(128)], moving v_hi [j, 2, h] -> og
    [i(128), 1024] PSUM: cols 0:768 numerator, col 768 denominator.
PSUM (8 banks): scores 2x[128,1024] (4) + og 2x[128,1024] (4).
"""

import math
from contextlib import ExitStack
from functools import lru_cache

import numpy as np

import concourse.bacc as bacc
import concourse.bass as bass
import concourse.tile as tile
from concourse import mybir
from concourse.bass_utils import run_bass_kernel_spmd

B, N, C = 4, 4096, 768
H = 768          # head dim (== C)
H3 = 3 * H
NCORES = 8
NK = N // 2      # keys (and locally projected rows) per core
DT = mybir.dt.float16
F8 = mybir.dt.float8e4
F32 = mybir.dt.float32
SCALE = 1.0 / math.sqrt(H)
DR = mybir.MatmulPerfMode.DoubleRow

CT = C // 128    # 6 contraction tiles (c) for the projection
HT = H // 128    # 6 head tiles (h)
HP = HT // 2     # 3 DoubleRow head-tile pairs
JT = NK // 128   # 16 key tiles (j) per core
JP = JT // 2     # 8 DoubleRow key-tile pairs
RB = 4           # r-blocks of 512 over this core's 2048 local rows
RBS = 512
IBP = 4          # i-block pairs of 1024 queries (attention)
IPS = N // IBP   # 1024
ITS = IPS // 128  # 8 i-tiles of 128 per i-block pair

PAIR_GROUPS = [[0, 1], [2, 3], [4, 5], [6, 7]]


def build_program():
    nc = bacc.Bacc(
        "TRN2",
        target_bir_lowering=False,
        debug=False,
        enable_asserts=False,
        num_devices=NCORES,
    )
    xT_d = nc.dram_tensor("xT", [C, NK], DT, kind="ExternalInput").ap()
    w_d = nc.dram_tensor("w", [C, H3], DT, kind="ExternalInput").ap()
    bqk_d = nc.dram_tensor("bqk", [128, 2 * HT], F32, kind="ExternalInput").ap()
    bv_d = nc.dram_tensor("bv", [128, H], F32, kind="ExternalInput").ap()
    # out[:, 0:768] = sum_j p'_j vhi_j  (numerator partial, minus colsum)
    # out[:, 768]   = sum_j p'_j       (denominator - 2048)
    out_d = nc.dram_tensor("out", [N, H + 1], DT, kind="ExternalOutput").ap()
    colsum_d = nc.dram_tensor("colsum", [1, H], F32, kind="ExternalOutput").ap()

    with tile.TileContext(nc) as tc:
        with ExitStack() as ctx:
            persist = ctx.enter_context(tc.tile_pool(name="persist", bufs=1))
            dram = ctx.enter_context(tc.tile_pool(name="dram", bufs=1,
                                                  space="DRAM"))

            qT8 = persist.tile([128, HT, N], F8, tag="qT8")
            kT8 = persist.tile([128, HT, NK], F8, tag="kT8")
            vhi = persist.tile([128, JT, H], F8, tag="vhi")
            vlo = persist.tile([128, JT, H], F8, tag="vlo")
            pp8 = persist.tile([128, JT, IPS], F8, tag="pp8")
            ones8 = persist.tile([128, 2, 16], F8, tag="ones8")
            bqk = persist.tile([128, 2 * HT], F32, tag="bqk")
            bvb = persist.tile([128, H], F32, tag="bvb")

            qloc = dram.tile([128, HT, NK], F8, tag="qloc")
            qgth = dram.tile([2, 128, HT, NK], F8, tag="qgth")

            # ---- Phase 1: QKV projection of the 2048 local rows ----
            with tc.tile_pool(name="wpool", bufs=1) as wpool, \
                 tc.tile_pool(name="xpool", bufs=3 * CT) as xpool, \
                 tc.tile_pool(name="vtmp", bufs=2) as vtmp, \
                 tc.tile_pool(name="pj", bufs=4, space="PSUM") as pj, \
                 tc.tile_pool(name="pv", bufs=2, space="PSUM") as pv:

                ws = [wpool.tile([128, H3], DT, tag=f"w{t}", name=f"w{t}")
                      for t in range(CT)]

                def load_xt(rb):
                    r0 = rb * RBS
                    tiles = []
                    for ct in range(CT):
                        t = xpool.tile([128, RBS], DT, tag="xt", name=f"xt{rb}_{ct}")
                        nc.sync.dma_start(
                            out=t, in_=xT_d[ct * 128:(ct + 1) * 128, r0:r0 + RBS])
                        tiles.append(t)
                    return tiles

                # DMA issue order = need order: q-projection h-tile-0 columns
                # of W interleaved with the first r-block's x, then biases,
                # then the rest of W.
                xts = [None] * RB
                xt0 = []
                for ct in range(CT):
                    nc.sync.dma_start(
                        out=ws[ct][:, 0:128],
                        in_=w_d[ct * 128:(ct + 1) * 128, 0:128])
                    t = xpool.tile([128, RBS], DT, tag="xt", name=f"xt0_{ct}")
                    nc.sync.dma_start(out=t, in_=xT_d[ct * 128:(ct + 1) * 128, 0:RBS])
                    xt0.append(t)
                xts[0] = xt0
                for ht in range(1, HT):
                    for ct in range(CT):
                        nc.sync.dma_start(
                            out=ws[ct][:, ht * 128:(ht + 1) * 128],
                            in_=w_d[ct * 128:(ct + 1) * 128, ht * 128:(ht + 1) * 128])
                    if ht == 1:
                        nc.sync.dma_start(out=bqk, in_=bqk_d)
                for ct in range(CT):
                    nc.sync.dma_start(out=ws[ct][:, H:2 * H],
                                      in_=w_d[ct * 128:(ct + 1) * 128, H:2 * H])
                    nc.sync.dma_start(out=ws[ct][:, 2 * H:H3],
                                      in_=w_d[ct * 128:(ct + 1) * 128, 2 * H:H3])
                nc.sync.dma_start(out=bvb, in_=bv_d)

                # PE warm-up: ~3.4us of junk matmuls (no DMA deps) so the
                # HAM clock-gate reaches full rate while the first x/W DMAs
                # are still in flight.
                warm_l = xpool.tile([128, 128], DT, tag="warml", name="warml")
                warm_r = xpool.tile([128, 512], DT, tag="warmr", name="warmr")
                nc.vector.memset(warm_l, 0.0)
                nc.vector.memset(warm_r, 0.0)
                nc.vector.memset(ones8, 1.0)
                for i in range(16):
                    wp = pj.tile([128, RBS], F32, tag="pj", name=f"warm{i}")
                    nc.tensor.matmul(wp, warm_l, warm_r, start=True, stop=True)

                for rb in range(RB):
                    r0 = rb * RBS
                    if rb + 1 < RB:
                        xts[rb + 1] = load_xt(rb + 1)
                    xt = xts[rb]

                    # q first: its staging DMAs gate the AllGather
                    for (wofs, dst) in ((0, qT8), (H, kT8)):
                        for ht in range(HT):
                            ps = pj.tile([128, RBS], F32, tag="pj")
                            for ct in range(CT):
                                nc.tensor.matmul(
                                    ps,
                                    ws[ct][:, wofs + ht * 128: wofs + (ht + 1) * 128],
                                    xt[ct],
                                    start=(ct == 0), stop=(ct == CT - 1),
                                )
                            bcol = (0 if wofs == 0 else HT) + ht
                            nc.scalar.activation(
                                out=dst[:, ht, r0:r0 + RBS],
                                in_=ps,
                                func=mybir.ActivationFunctionType.Identity,
                                bias=bqk[:, bcol:bcol + 1],
                            )
                            if wofs == 0:
                                nc.gpsimd.dma_start(
                                    out=qloc[:, ht, r0:r0 + RBS],
                                    in_=dst[:, ht, r0:r0 + RBS])

                    for j in range(RBS // 128):
                        jt = rb * (RBS // 128) + j
                        ps = pv.tile([128, H], F32, tag="pv")
                        for ct in range(CT):
                            xs = xt[ct][:, j * 128:(j + 1) * 128]
                            nc.tensor.matmul(
                                ps[:, 0:512], xs, ws[ct][:, 2 * H: 2 * H + 512],
                                start=(ct == 0), stop=(ct == CT - 1))
                            nc.tensor.matmul(
                                ps[:, 512:H], xs, ws[ct][:, 2 * H + 512: 3 * H],
                                start=(ct == 0), stop=(ct == CT - 1))
                        s1 = vtmp.tile([128, H], DT, tag="s1")
                        nc.vector.tensor_add(s1, ps, bvb)
                        nc.vector.tensor_copy(out=vhi[:, jt, :], in_=s1)
                        nc.vector.tensor_sub(vlo[:, jt, :], s1, vhi[:, jt, :])

                    if rb == RB - 1:
                        # All q staged: exchange q halves across the pair.
                        # Gather order = [rows 0:2048, rows 2048:4096] in
                        # ORIGINAL batch coordinates, identical on both
                        # cores (core kh's local rows are kh*2048:+2048).
                        nc.gpsimd.collective_compute(
                            "AllGather",
                            mybir.AluOpType.bypass,
                            replica_groups=PAIR_GROUPS,
                            ins=[qloc.opt()],
                            outs=[qgth.opt()],
                        )
                        # read back per ht-pair so QK hp0 can start early
                        for hp in range(HP):
                            for half in range(2):
                                nc.gpsimd.dma_start(
                                    out=qT8[:, 2 * hp:2 * hp + 2,
                                            half * NK:(half + 1) * NK],
                                    in_=qgth[half, :, 2 * hp:2 * hp + 2, :])

                # colsum_v8[h] = sum_j (vhi + vlo)[j, h] via ones-stationary
                # DoubleRow matmuls; reuses the pv psum banks.
                csum = vtmp.tile([1, H], F32, tag="csum")
                for seg0, seg1 in ((0, 512), (512, H)):
                    cs_ps = pv.tile([1, 512], F32, tag="pv", name=f"cs{seg0}")
                    for jp in range(JP):
                        for vi, vt in enumerate((vhi, vlo)):
                            nc.tensor.matmul(
                                cs_ps[:, 0:seg1 - seg0],
                                ones8[:, :, 0:1],
                                vt[:, 2 * jp:2 * jp + 2, seg0:seg1],
                                start=(jp == 0 and vi == 0),
                                stop=(jp == JP - 1 and vi == 1),
                                perf_mode=DR,
                            )
                    nc.vector.tensor_copy(out=csum[:, seg0:seg1],
                                          in_=cs_ps[:, 0:seg1 - seg0])
                nc.sync.dma_start(out=colsum_d, in_=csum)

            # ---- Phase 2: attention (partial sums over this core's keys) ----
            with tc.tile_pool(name="p16pool", bufs=2) as p16pool, \
                 tc.tile_pool(name="opool", bufs=4) as opool, \
                 tc.tile_pool(name="ps_s", bufs=2, space="PSUM") as ps_s, \
                 tc.tile_pool(name="ps_o", bufs=2, space="PSUM") as ps_o:

                for ibp in range(IBP):
                    i0 = ibp * IPS
                    # Stage A: scores + exp + p' for this i-block pair
                    for jt in range(JT):
                        sps = ps_s.tile([128, IPS], F32, tag="s")
                        for hp in range(HP):
                            kslice = kT8[:, 2 * hp:2 * hp + 2,
                                         jt * 128:(jt + 1) * 128]
                            for half in range(2):
                                nc.tensor.matmul(
                                    sps[:, half * 512:(half + 1) * 512],
                                    kslice,
                                    qT8[:, 2 * hp:2 * hp + 2,
                                        i0 + half * 512: i0 + (half + 1) * 512],
                                    start=(hp == 0), stop=(hp == HP - 1),
                                    perf_mode=DR,
                                )
                        p16 = p16pool.tile([128, IPS], DT, tag="p16")
                        nc.scalar.activation(
                            out=p16, in_=sps,
                            func=mybir.ActivationFunctionType.Exp,
                            scale=SCALE,
                        )
                        nc.vector.tensor_scalar_sub(pp8[:, jt, :], p16, 1.0)

                    # Stage B: PV per i-tile; den' into column 768.
                    # p'@vlo is dropped: both factors are small and
                    # uncorrelated (~6e-4 of the output); vlo still feeds the
                    # colsum correction, so v carries no bias.
                    for it in range(ITS):
                        og = ps_o.tile([128, IPS], F32, tag="og",
                                       name=f"og{ibp}_{it}")
                        for jp in range(JP):
                            pslice = pp8[:, 2 * jp:2 * jp + 2,
                                         it * 128:(it + 1) * 128]
                            st = (jp == 0)
                            sp = (jp == JP - 1)
                            nc.tensor.matmul(
                                og[:, 0:512], pslice,
                                vhi[:, 2 * jp:2 * jp + 2, 0:512],
                                start=st, stop=sp, perf_mode=DR)
                            nc.tensor.matmul(
                                og[:, 512:H], pslice,
                                vhi[:, 2 * jp:2 * jp + 2, 512:H],
                                start=st, stop=sp, perf_mode=DR)
                            # NO start=True here: PSUM zeroing is bank-granular
                            # (2KB); the hi-seg2 matmul's start already marks
                            # the whole bank pending-zero, which zeroes these
                            # bytes on first touch. A start here would re-mark
                            # the bank and wipe hi-seg2's jp0 contribution.
                            nc.tensor.matmul(
                                og[:, H:H + 1], pslice,
                                ones8[:, :, 0:1],
                                start=False, stop=sp, perf_mode=DR,
                                skip_group_check=True)
                        ot = opool.tile([128, H + 1], DT, tag="ot",
                                        name=f"ot{ibp}_{it}")
                        nc.scalar.activation(
                            out=ot[:, 0:384], in_=og[:, 0:384],
                            func=mybir.ActivationFunctionType.Copy)
                        nc.vector.tensor_copy(out=ot[:, 384:H + 1],
                                              in_=og[:, 384:H + 1])
                        r0 = i0 + it * 128
                        dma = nc.scalar.dma_start if it % 2 else nc.sync.dma_start
                        dma(out=out_d[r0:r0 + 128, :], in_=ot)
    nc.compile()
    return nc


@lru_cache(maxsize=1)
def _cached_program():
    return build_program()


def _prep_in_maps(x, W_qkv, b_qkv):
    x = np.asarray(x, dtype=np.float32)
    W_qkv = np.asarray(W_qkv, dtype=np.float32)
    b_qkv = np.asarray(b_qkv, dtype=np.float32)
    w16 = W_qkv.astype(np.float16)
    bq = b_qkv[0:H].astype(np.float32).reshape(HT, 128).T    # [128, HT]
    bk = b_qkv[H:2 * H].astype(np.float32).reshape(HT, 128).T
    bqk = np.ascontiguousarray(np.concatenate([bq, bk], axis=1))  # [128, 2*HT]
    bv = np.ascontiguousarray(
        np.broadcast_to(b_qkv[2 * H:3 * H].astype(np.float32), (128, H)))

    in_maps = []
    for core in range(NCORES):
        b, kh = core // 2, core % 2
        # this core's local rows = its key half, in original order
        xb = x[b][kh * NK:(kh + 1) * NK]          # [NK, C]
        xT = np.ascontiguousarray(xb.T).astype(np.float16)
        in_maps.append({"xT": xT, "w": w16, "bqk": bqk, "bv": bv})
    return in_maps


def _combine(results):
    out = np.empty((B, N, C), dtype=np.float32)
    for b in range(B):
        acc_n = None
        acc_d = None
        for kh in (0, 1):
            r = results[2 * b + kh]
            o = np.asarray(r["out"], dtype=np.float32)      # [N, H+1]
            cs = np.asarray(r["colsum"], dtype=np.float32)  # [1, H]
            num = o[:, 0:H] + cs                            # add colsum_v8
            den = o[:, H] + float(NK)                       # add sum_j 1
            acc_n = num if acc_n is None else acc_n + num
            acc_d = den if acc_d is None else acc_d + den
        out[b] = acc_n / acc_d[:, None]
    return out


def kernel(x, W_qkv, b_qkv):
    nc = _cached_program()
    in_maps = _prep_in_maps(x, W_qkv, b_qkv)
    res = run_bass_kernel_spmd(nc, in_maps, core_ids=list(range(NCORES)))
    return _combine(res.results)


# revision 9
# speedup vs baseline: 1.5954x; 1.0924x over previous
"""Fused single-head attention (QKV projection + softmax(QK^T)V) on 8 trn2 cores.

Problem (hardcoded): x [4, 4096, 768] f32, W_qkv [768, 2304] f32, b_qkv [2304] f32.
  qkv = x @ W_qkv + b_qkv ; q,k,v = split(qkv, 3)
  out = softmax(q k^T / sqrt(768)) v          -> [4, 4096, 768] f32

Sharding: batch (4) x key-halves (2) -> 8 cores. Each core receives ONLY its
key half of x (xT [768, 2048] fp16, rows kh*2048:(kh+1)*2048 of the batch),
projects q/k/v for those 2048 rows, then an AllGather over the core pair
rebuilds the full q (4096 queries, ORIGINAL row order, identical on both
cores). Each core computes PARTIAL attention sums over its 2048 keys; the
host combines pairs: (n0 + n1) / (d0 + d1). No max-subtraction: scores O(1).

fp8 strategy (PE DoubleRow = 256-deep contraction per pass, 2x fp16 FLOPs;
matmul cost = out-columns x 1 cycle regardless of dtype):
  - Projection fp16 (fp8 proj error 2.8e-2 > 2e-2 gate).
  - QK^T fp8 DoubleRow (q,k e4m3 via the projection bias-activation).
  - PV fp8 DoubleRow with p' = fp8(exp(s)-1) (p~1 so centering shrinks the
    fp8 error ~3x; numerator correction colsum_v8 = sum_j v8_j added on
    host). v8 = v_hi + v_lo fp8 pair, but p'@v_lo is DROPPED (product of two
    small uncorrelated factors, ~6e-4 of output); v_lo feeds only colsum so
    v carries no bias. Simulated end-to-end error 1.62e-2 (gate 2e-2).
  - Softmax denominator: one extra DoubleRow matmul per (i-tile, j-pair)
    reusing the p' stationary with a ones moving vector -> output column 768;
    host adds back 2048.

On-chip layout:
  - qT8 [h(128), ht(6), i(4096)], kT8 [h, ht, j(2048)] fp8, h-major;
    DoubleRow pairs adjacent ht tiles.
  - scores transposed: s[j(128), i] = kT-tile.T @ qT -> PSUM [128, 1024]
    (i-block pair so the k stationary is reused), exp on ScalarE -> p16,
    p' = p16 - 1 on VectorE -> pp8 [j(128), jt(16), i(1024)] fp8.
  - PV: stationary p' [j, 2, i-tile(128)], moving v_hi [j, 2, h] -> og
    [i(128), 1024] PSUM: cols 0:768 numerator, col 768 denominator.
PSUM (8 banks): scores 2x[128,1024] (4) + og 2x[128,1024] (4).
"""

import math
from contextlib import ExitStack
from functools import lru_cache

import numpy as np

import concourse.bacc as bacc
import concourse.bass as bass
import concourse.tile as tile
from concourse import mybir
from concourse.bass_utils import run_bass_kernel_spmd

B, N, C = 4, 4096, 768
H = 768          # head dim (== C)
H3 = 3 * H
NCORES = 8
NK = N // 2      # keys (and locally projected rows) per core
DT = mybir.dt.float16
F8 = mybir.dt.float8e4
F32 = mybir.dt.float32
SCALE = 1.0 / math.sqrt(H)
DR = mybir.MatmulPerfMode.DoubleRow

CT = C // 128    # 6 contraction tiles (c) for the projection
HT = H // 128    # 6 head tiles (h)
HP = HT // 2     # 3 DoubleRow head-tile pairs
JT = NK // 128   # 16 key tiles (j) per core
JP = JT // 2     # 8 DoubleRow key-tile pairs
RB = 4           # r-blocks of 512 over this core's 2048 local rows
RBS = 512
IBP = 4          # i-block pairs of 1024 queries (attention)
IPS = N // IBP   # 1024
ITS = IPS // 128  # 8 i-tiles of 128 per i-block pair

PAIR_GROUPS = [[0, 1], [2, 3], [4, 5], [6, 7]]


def build_program():
    nc = bacc.Bacc(
        "TRN2",
        target_bir_lowering=False,
        debug=False,
        enable_asserts=False,
        num_devices=NCORES,
    )
    xT_d = nc.dram_tensor("xT", [C, NK], DT, kind="ExternalInput").ap()
    w_d = nc.dram_tensor("w", [C, H3], DT, kind="ExternalInput").ap()
    bqk_d = nc.dram_tensor("bqk", [128, 2 * HT], F32, kind="ExternalInput").ap()
    bv_d = nc.dram_tensor("bv", [128, H], F32, kind="ExternalInput").ap()
    # out[:, 0:768] = sum_j p'_j vhi_j  (numerator partial, minus colsum)
    # out[:, 768]   = sum_j p'_j       (denominator - 2048)
    out_d = nc.dram_tensor("out", [N, H + 1], DT, kind="ExternalOutput").ap()
    colsum_d = nc.dram_tensor("colsum", [1, H], F32, kind="ExternalOutput").ap()

    with tile.TileContext(nc) as tc:
        with ExitStack() as ctx:
            persist = ctx.enter_context(tc.tile_pool(name="persist", bufs=1))
            dram = ctx.enter_context(tc.tile_pool(name="dram", bufs=1,
                                                  space="DRAM"))

            qT8 = persist.tile([128, HT, N], F8, tag="qT8")
            kT8 = persist.tile([128, HT, NK], F8, tag="kT8")
            vhi = persist.tile([128, JT, H], F8, tag="vhi")
            vlo = persist.tile([128, JT, H], F8, tag="vlo")
            pp8 = persist.tile([128, JT, IPS], F8, tag="pp8")
            ones8 = persist.tile([128, 2, 16], F8, tag="ones8")
            bqk = persist.tile([128, 2 * HT], F32, tag="bqk")
            bvb = persist.tile([128, H], F32, tag="bvb")

            # q exchanged in two 1024-row chunks so the collectives overlap
            # the tail of the projection
            NC2 = NK // 2
            qloc = [dram.tile([128, HT, NC2], F8, tag=f"qloc{c}",
                              name=f"qloc{c}") for c in range(2)]
            qgth = [dram.tile([2, 128, HT, NC2], F8, tag=f"qgth{c}",
                              name=f"qgth{c}") for c in range(2)]

            # ---- Phase 1: QKV projection of the 2048 local rows ----
            with tc.tile_pool(name="wpool", bufs=1) as wpool, \
                 tc.tile_pool(name="xpool", bufs=3 * CT) as xpool, \
                 tc.tile_pool(name="vtmp", bufs=2) as vtmp, \
                 tc.tile_pool(name="pj", bufs=4, space="PSUM") as pj, \
                 tc.tile_pool(name="pv", bufs=2, space="PSUM") as pv:

                ws = [wpool.tile([128, H3], DT, tag=f"w{t}", name=f"w{t}")
                      for t in range(CT)]

                def load_xt(rb):
                    r0 = rb * RBS
                    tiles = []
                    for ct in range(CT):
                        t = xpool.tile([128, RBS], DT, tag="xt", name=f"xt{rb}_{ct}")
                        nc.sync.dma_start(
                            out=t, in_=xT_d[ct * 128:(ct + 1) * 128, r0:r0 + RBS])
                        tiles.append(t)
                    return tiles

                # DMA issue order = need order: q-projection h-tile-0 columns
                # of W interleaved with the first r-block's x, then biases,
                # then the rest of W.
                xts = [None] * RB
                xt0 = []
                for ct in range(CT):
                    nc.sync.dma_start(
                        out=ws[ct][:, 0:128],
                        in_=w_d[ct * 128:(ct + 1) * 128, 0:128])
                    t = xpool.tile([128, RBS], DT, tag="xt", name=f"xt0_{ct}")
                    nc.sync.dma_start(out=t, in_=xT_d[ct * 128:(ct + 1) * 128, 0:RBS])
                    xt0.append(t)
                xts[0] = xt0
                for ht in range(1, HT):
                    for ct in range(CT):
                        nc.sync.dma_start(
                            out=ws[ct][:, ht * 128:(ht + 1) * 128],
                            in_=w_d[ct * 128:(ct + 1) * 128, ht * 128:(ht + 1) * 128])
                    if ht == 1:
                        nc.sync.dma_start(out=bqk, in_=bqk_d)
                for ct in range(CT):
                    nc.sync.dma_start(out=ws[ct][:, H:2 * H],
                                      in_=w_d[ct * 128:(ct + 1) * 128, H:2 * H])
                    nc.sync.dma_start(out=ws[ct][:, 2 * H:H3],
                                      in_=w_d[ct * 128:(ct + 1) * 128, 2 * H:H3])
                nc.sync.dma_start(out=bvb, in_=bv_d)

                # PE warm-up: ~3.4us of junk matmuls (no DMA deps) so the
                # HAM clock-gate reaches full rate while the first x/W DMAs
                # are still in flight.
                warm_l = xpool.tile([128, 128], DT, tag="warml", name="warml")
                warm_r = xpool.tile([128, 512], DT, tag="warmr", name="warmr")
                nc.vector.memset(warm_l, 0.0)
                nc.vector.memset(warm_r, 0.0)
                nc.vector.memset(ones8, 1.0)
                for i in range(16):
                    wp = pj.tile([128, RBS], F32, tag="pj", name=f"warm{i}")
                    nc.tensor.matmul(wp, warm_l, warm_r, start=True, stop=True)

                for rb in range(RB):
                    r0 = rb * RBS
                    if rb + 1 < RB:
                        xts[rb + 1] = load_xt(rb + 1)
                    xt = xts[rb]

                    # q first: its staging DMAs gate the AllGather
                    for (wofs, dst) in ((0, qT8), (H, kT8)):
                        for ht in range(HT):
                            ps = pj.tile([128, RBS], F32, tag="pj")
                            for ct in range(CT):
                                nc.tensor.matmul(
                                    ps,
                                    ws[ct][:, wofs + ht * 128: wofs + (ht + 1) * 128],
                                    xt[ct],
                                    start=(ct == 0), stop=(ct == CT - 1),
                                )
                            bcol = (0 if wofs == 0 else HT) + ht
                            nc.scalar.activation(
                                out=dst[:, ht, r0:r0 + RBS],
                                in_=ps,
                                func=mybir.ActivationFunctionType.Identity,
                                bias=bqk[:, bcol:bcol + 1],
                            )
                            if wofs == 0:
                                nc.gpsimd.dma_start(
                                    out=qloc[rb // 2][:, ht,
                                                      (r0 % NC2):(r0 % NC2) + RBS],
                                    in_=dst[:, ht, r0:r0 + RBS])

                    for j in range(RBS // 128):
                        jt = rb * (RBS // 128) + j
                        ps = pv.tile([128, H], F32, tag="pv")
                        for ct in range(CT):
                            xs = xt[ct][:, j * 128:(j + 1) * 128]
                            nc.tensor.matmul(
                                ps[:, 0:512], xs, ws[ct][:, 2 * H: 2 * H + 512],
                                start=(ct == 0), stop=(ct == CT - 1))
                            nc.tensor.matmul(
                                ps[:, 512:H], xs, ws[ct][:, 2 * H + 512: 3 * H],
                                start=(ct == 0), stop=(ct == CT - 1))
                        s1 = vtmp.tile([128, H], DT, tag="s1")
                        nc.vector.tensor_add(s1, ps, bvb)
                        nc.vector.tensor_copy(out=vhi[:, jt, :], in_=s1)
                        nc.vector.tensor_sub(vlo[:, jt, :], s1, vhi[:, jt, :])

                    if rb % 2 == 1:
                        # Chunk (rb//2) of local q fully staged: exchange it
                        # across the pair while the projection continues.
                        # Chunk c gathers ORIGINAL query rows
                        # [c*1024:(c+1)*1024] (from the even core) and
                        # [2048+c*1024:...] (from the odd core) — identical
                        # result on both cores, no un-rotation anywhere.
                        c = rb // 2
                        nc.gpsimd.collective_compute(
                            "AllGather",
                            mybir.AluOpType.bypass,
                            replica_groups=PAIR_GROUPS,
                            ins=[qloc[c].opt()],
                            outs=[qgth[c].opt()],
                        )
                        # read back per ht-pair (QK hp0 starts early); spread
                        # across the otherwise-idle sync/scalar DMA queues
                        for hp in range(HP):
                            for half in range(2):
                                dma = (nc.sync.dma_start if (hp + half) % 2
                                       else nc.scalar.dma_start)
                                dma(out=qT8[:, 2 * hp:2 * hp + 2,
                                            half * NK + c * NC2:
                                            half * NK + (c + 1) * NC2],
                                    in_=qgth[c][half, :, 2 * hp:2 * hp + 2, :])

                # colsum_v8[h] = sum_j (vhi + vlo)[j, h] via ones-stationary
                # DoubleRow matmuls; reuses the pv psum banks.
                csum = vtmp.tile([1, H], F32, tag="csum")
                for seg0, seg1 in ((0, 512), (512, H)):
                    cs_ps = pv.tile([1, 512], F32, tag="pv", name=f"cs{seg0}")
                    for jp in range(JP):
                        for vi, vt in enumerate((vhi, vlo)):
                            nc.tensor.matmul(
                                cs_ps[:, 0:seg1 - seg0],
                                ones8[:, :, 0:1],
                                vt[:, 2 * jp:2 * jp + 2, seg0:seg1],
                                start=(jp == 0 and vi == 0),
                                stop=(jp == JP - 1 and vi == 1),
                                perf_mode=DR,
                            )
                    nc.vector.tensor_copy(out=csum[:, seg0:seg1],
                                          in_=cs_ps[:, 0:seg1 - seg0])
                nc.sync.dma_start(out=colsum_d, in_=csum)

            # ---- Phase 2: attention (partial sums over this core's keys) ----
            with tc.tile_pool(name="p16pool", bufs=2) as p16pool, \
                 tc.tile_pool(name="opool", bufs=4) as opool, \
                 tc.tile_pool(name="ps_s", bufs=2, space="PSUM") as ps_s, \
                 tc.tile_pool(name="ps_o", bufs=2, space="PSUM") as ps_o:

                # [0, 2] use exchange chunk 0, [1, 3] chunk 1 — chunk-0 pairs
                # first so phase 2 never waits on the second collective
                for ibp in (0, 2, 1, 3):
                    i0 = ibp * IPS
                    # Stage A: scores + exp + p' for this i-block pair
                    for jt in range(JT):
                        sps = ps_s.tile([128, IPS], F32, tag="s")
                        for hp in range(HP):
                            kslice = kT8[:, 2 * hp:2 * hp + 2,
                                         jt * 128:(jt + 1) * 128]
                            for half in range(2):
                                nc.tensor.matmul(
                                    sps[:, half * 512:(half + 1) * 512],
                                    kslice,
                                    qT8[:, 2 * hp:2 * hp + 2,
                                        i0 + half * 512: i0 + (half + 1) * 512],
                                    start=(hp == 0), stop=(hp == HP - 1),
                                    perf_mode=DR,
                                )
                        p16 = p16pool.tile([128, IPS], DT, tag="p16")
                        nc.scalar.activation(
                            out=p16, in_=sps,
                            func=mybir.ActivationFunctionType.Exp,
                            scale=SCALE,
                        )
                        nc.vector.tensor_scalar_sub(pp8[:, jt, :], p16, 1.0)

                    # Stage B: PV per i-tile; den' into column 768.
                    # p'@vlo is dropped: both factors are small and
                    # uncorrelated (~6e-4 of the output); vlo still feeds the
                    # colsum correction, so v carries no bias.
                    for it in range(ITS):
                        og = ps_o.tile([128, IPS], F32, tag="og",
                                       name=f"og{ibp}_{it}")
                        for jp in range(JP):
                            pslice = pp8[:, 2 * jp:2 * jp + 2,
                                         it * 128:(it + 1) * 128]
                            st = (jp == 0)
                            sp = (jp == JP - 1)
                            nc.tensor.matmul(
                                og[:, 0:512], pslice,
                                vhi[:, 2 * jp:2 * jp + 2, 0:512],
                                start=st, stop=sp, perf_mode=DR)
                            nc.tensor.matmul(
                                og[:, 512:H], pslice,
                                vhi[:, 2 * jp:2 * jp + 2, 512:H],
                                start=st, stop=sp, perf_mode=DR)
                            # NO start=True here: PSUM zeroing is bank-granular
                            # (2KB); the hi-seg2 matmul's start already marks
                            # the whole bank pending-zero, which zeroes these
                            # bytes on first touch. A start here would re-mark
                            # the bank and wipe hi-seg2's jp0 contribution.
                            nc.tensor.matmul(
                                og[:, H:H + 1], pslice,
                                ones8[:, :, 0:1],
                                start=False, stop=sp, perf_mode=DR,
                                skip_group_check=True)
                        ot = opool.tile([128, H + 1], DT, tag="ot",
                                        name=f"ot{ibp}_{it}")
                        nc.scalar.activation(
                            out=ot[:, 0:384], in_=og[:, 0:384],
                            func=mybir.ActivationFunctionType.Copy)
                        nc.vector.tensor_copy(out=ot[:, 384:H + 1],
                                              in_=og[:, 384:H + 1])
                        r0 = i0 + it * 128
                        dma = nc.scalar.dma_start if it % 2 else nc.sync.dma_start
                        dma(out=out_d[r0:r0 + 128, :], in_=ot)
    nc.compile()
    return nc


@lru_cache(maxsize=1)
def _cached_program():
    return build_program()


def _prep_in_maps(x, W_qkv, b_qkv):
    x = np.asarray(x, dtype=np.float32)
    W_qkv = np.asarray(W_qkv, dtype=np.float32)
    b_qkv = np.asarray(b_qkv, dtype=np.float32)
    w16 = W_qkv.astype(np.float16)
    bq = b_qkv[0:H].astype(np.float32).reshape(HT, 128).T    # [128, HT]
    bk = b_qkv[H:2 * H].astype(np.float32).reshape(HT, 128).T
    bqk = np.ascontiguousarray(np.concatenate([bq, bk], axis=1))  # [128, 2*HT]
    bv = np.ascontiguousarray(
        np.broadcast_to(b_qkv[2 * H:3 * H].astype(np.float32), (128, H)))

    in_maps = []
    for core in range(NCORES):
        b, kh = core // 2, core % 2
        # this core's local rows = its key half, in original order
        xb = x[b][kh * NK:(kh + 1) * NK]          # [NK, C]
        xT = np.ascontiguousarray(xb.T).astype(np.float16)
        in_maps.append({"xT": xT, "w": w16, "bqk": bqk, "bv": bv})
    return in_maps


def _combine(results):
    out = np.empty((B, N, C), dtype=np.float32)
    for b in range(B):
        acc_n = None
        acc_d = None
        for kh in (0, 1):
            r = results[2 * b + kh]
            o = np.asarray(r["out"], dtype=np.float32)      # [N, H+1]
            cs = np.asarray(r["colsum"], dtype=np.float32)  # [1, H]
            num = o[:, 0:H] + cs                            # add colsum_v8
            den = o[:, H] + float(NK)                       # add sum_j 1
            acc_n = num if acc_n is None else acc_n + num
            acc_d = den if acc_d is None else acc_d + den
        out[b] = acc_n / acc_d[:, None]
    return out


def kernel(x, W_qkv, b_qkv):
    nc = _cached_program()
    in_maps = _prep_in_maps(x, W_qkv, b_qkv)
    res = run_bass_kernel_spmd(nc, in_maps, core_ids=list(range(NCORES)))
    return _combine(res.results)
